# revision 30
# baseline (speedup 1.0000x reference)
"""Cross-attention layer kernel for 8 Trainium2 NeuronCores.

Reference computation (fp32, D=1024, S=2048, B=4):
    q = x @ Wq.T + bq ; k = x @ Wk.T + bk ; v = x @ Wv.T + bv
    attn = softmax(q @ k.T / 32)
    vision = attn @ v                      # [B,S,D]
    text   = attn.T @ x                    # [B,S,D]

Algebraic restructure (all projections folded):
    scores = x_q M x^T + u[q] + v[k] + c   with M = Wq^T Wk (host),
        u = x_q (Wq^T bk), v = x (Wk^T^T bq) = x (bq@Wk), c = bq.bk
    attn   = exp(s/32) row-normalized; the column factor exp(v[k]/32)
        is applied on the Vector engine via a broadcast tile, the row
        factor exp((u[q]+c)/32) via the Exp activation's bias input.
    vision = (attn @ x) @ Wv^T + bv        # Z^T = x^T-contraction form
    text   = attn^T @ x_q                  # partial, host sums the pair

Sharding: core c handles batch b=c//2, query-half h=c%2 (1024 queries,
all 2048 keys).  Key order inside a core is [own half | other half] so
the program is static; the host permutes inputs and un-permutes text.

Per-core device work is 1024 N=512 bf16 matmuls (8.6 GMAC) and zero
PE transposes: x^T comes from the host, attn^T (P^T) from the DMA XBAR
transpose (2-byte dtype), and every output is produced in its natural
orientation (text[k,d] via P as lhsT, vision[q,e] via Z^T as lhsT).
All tensors are SBUF-resident bf16 (~172 KB/partition), no DRAM spills.

SBUF slot reuse: gz holds g^T = (x_q M)^T until scores are done, then
Z^T; mw holds M until g^T is done, then Wv^T.
"""

import sys

import numpy as np

try:
    import concourse.bass as bass
except ImportError:  # pragma: no cover - grading env should have it on path
    sys.path.insert(0, "/opt/trn_rl_repo")
    import concourse.bass as bass

import ml_dtypes
import concourse.mybir as mybir
import concourse.tile as tile
from concourse import bacc
from concourse.bass_utils import run_bass_kernel_spmd

F32 = mybir.dt.float32
BF16 = mybir.dt.bfloat16
BF16_NP = ml_dtypes.bfloat16

B = 4          # batches
S = 2048       # sequence length
D = 1024       # model dim
SH = S // 2    # queries per core
P = 128        # partitions
NT = D // P    # 8 tiles along d
NQ = SH // P   # 8 q-tiles per core
NK = S // P    # 16 k-tiles
NC = S // 512  # 4 512-chunks along k
SCALE = 1.0 / 32.0  # 1/sqrt(D)
N512 = 512


def build_program():
    nc = bacc.Bacc("TRN2", target_bir_lowering=False, debug=False, num_devices=8)

    xt_h = nc.dram_tensor("xt", [D, S], BF16, kind="ExternalInput")    # x^T, cols [own|other]
    xr_h = nc.dram_tensor("xr", [S, D], BF16, kind="ExternalInput")    # x rows [own|other]
    # M = Wq^T Wk and Wv^T, host-swizzled to partition-major layouts so
    # DMA packets are >=2 KB on both the DRAM and SBUF side.
    m_h = nc.dram_tensor("m", [P, D * NT], BF16, kind="ExternalInput")
    wvt_h = nc.dram_tensor("wvt", [P, D * NT], BF16, kind="ExternalInput")
    u_h = nc.dram_tensor("u", [P, NQ], F32, kind="ExternalInput")      # (u+c)/32, own qs
    phi_h = nc.dram_tensor("phi", [S], BF16, kind="ExternalInput")     # exp(v/32), key order
    bv_h = nc.dram_tensor("bv", [D], F32, kind="ExternalInput")

    vision_h = nc.dram_tensor("vision", [SH, D], BF16, kind="ExternalOutput")
    text_h = nc.dram_tensor("text", [S, D], BF16, kind="ExternalOutput")

    # tiled DRAM views
    xt_r = xt_h.ap().rearrange("(t p) k -> p t k", p=P)    # [128,8,2048]
    xr_r = xr_h.ap().rearrange("(i p) d -> p i d", p=P)    # [128,16,1024]
    m_r = m_h.ap().rearrange("p (e t c) -> e p t c", e=NT, t=NT)  # [8,128,8,128]
    wvt_r = wvt_h.ap().rearrange("p (t h c) -> p t h c", t=NT, h=NT)

    phi_ap = phi_h.ap()
    phi_bcast = bass.AP(tensor=phi_ap.tensor, offset=phi_ap.offset,
                        ap=[[0, P], phi_ap.ap[0]])         # [128,2048]
    bv_ap = bv_h.ap()
    bv_bcast = bass.AP(tensor=bv_ap.tensor, offset=bv_ap.offset,
                       ap=[[0, P], bv_ap.ap[0]])           # [128,1024]

    with tile.TileContext(nc) as tc:
        with (
            tc.tile_pool(name="singles", bufs=1) as singles,
            tc.tile_pool(name="t1pool", bufs=2) as t1pool,
            tc.tile_pool(name="stage", bufs=6) as stage,
            tc.tile_pool(name="psum", bufs=8, space="PSUM") as pp,
        ):
            # persistent SBUF tensors (bytes/partition)
            xT = singles.tile([P, NT, S], BF16)     # 32K  x^T [d, k]
            xrows = singles.tile([P, NK, D], BF16)  # 32K  x   [k, d]
            Psb = singles.tile([P, NQ, S], BF16)    # 32K  attn [q, k]
            PT = singles.tile([P, NK, SH], BF16)    # 32K  attn^T [k, q]
            gz = singles.tile([P, NT, SH], BF16)    # 16K  g^T [d', q] then Z^T [d, q]
            # M as [p, et, dt, c] (M[dt*128+p, et*128+c]); reused for
            # Wv^T as [p, dt, eh, el] (Wv^T[dt*128+p, eh*128+el]).
            mw = singles.tile([P, NT, NT, P], BF16)  # 16K
            phib = singles.tile([P, S], BF16)       # 4K
            bvb = singles.tile([P, D], F32)         # 4K
            u_sb = singles.tile([P, NQ], F32)
            l_sb = singles.tile([P, NQ], F32)
            r_sb = singles.tile([P, NQ], F32)

            # ---- input DMAs -------------------------------------------
            # All loads have >=2 KB packets.  x^T own-half rows split
            # across the SP and Activation queues so the first g^T chain
            # (M block 0 + 8 own x^T rows) is fed in ~3 us; M blocks on
            # gpsimd arrive at 2x the rate phase 1 consumes them.
            nc.sync.dma_start(out=mw[:, 0], in_=m_r[0])
            for dt in range(NT):
                eng = nc.sync if dt % 2 == 0 else nc.scalar
                eng.dma_start(out=xT[:, dt, 0:SH], in_=xt_r[:, dt, 0:SH])
            for et in range(1, 4):
                nc.gpsimd.dma_start(out=mw[:, et], in_=m_r[et])
            for et in range(4, NT):
                nc.sync.dma_start(out=mw[:, et], in_=m_r[et])
            for dt in range(NT):
                eng = nc.sync if dt % 2 == 0 else nc.scalar
                eng.dma_start(out=xT[:, dt, SH:S], in_=xt_r[:, dt, SH:S])
            nc.gpsimd.dma_start(out=u_sb, in_=u_h.ap())
            nc.gpsimd.dma_start(out=phib, in_=phi_bcast)
            nc.gpsimd.dma_start(out=bvb, in_=bv_bcast)
            for i0 in range(0, NK, 8):
                nc.gpsimd.dma_start(out=xrows[:, i0:i0 + 8, :],
                                    in_=xr_r[:, i0:i0 + 8, :])

            # ---- phase 1: g^T = (x_q M)^T  [128 matmuls] --------------
            # (the first chain paces itself with the DMA arrivals: the
            # dt-contraction order matches the x^T row load order, so the
            # p-state ramp hides under the input transfer time)
            for et in range(NT):
                for qc in range(2):
                    ps = pp.tile([P, N512], F32, tag="acc")
                    for dt in range(NT):
                        nc.tensor.matmul(
                            ps,
                            mw[:, et, dt, :],
                            xT[:, dt, qc * N512:(qc + 1) * N512],
                            start=(dt == 0), stop=(dt == NT - 1))
                    nc.scalar.activation(
                        gz[:, et, qc * N512:(qc + 1) * N512], ps,
                        mybir.ActivationFunctionType.Identity,
                        bias=0.0, scale=1.0)

            # Wv^T reuses M's slot: emitted only now so the tile
            # dependency tracker sequences it after phase 1's M reads.
            nc.gpsimd.dma_start(out=mw, in_=wvt_r)

            # ---- phase 2: scores + exp + col/row scaling  [256 mm] ----
            for j in range(NQ):
                for kc in range(NC):
                    ps = pp.tile([P, N512], F32, tag="acc")
                    for et in range(NT):
                        nc.tensor.matmul(
                            ps,
                            gz[:, et, j * P:(j + 1) * P],
                            xT[:, et, kc * N512:(kc + 1) * N512],
                            start=(et == 0), stop=(et == NT - 1))
                    nc.scalar.activation(
                        Psb[:, j, kc * N512:(kc + 1) * N512], ps,
                        mybir.ActivationFunctionType.Exp,
                        bias=u_sb[:, j:j + 1], scale=SCALE)
                # DVE: apply exp(v/32) column factor, row-normalize
                t1 = t1pool.tile([P, S], F32, tag="t1")
                nc.vector.tensor_mul(t1, Psb[:, j, :], phib)
                nc.vector.reduce_sum(out=l_sb[:, j:j + 1], in_=t1,
                                     axis=mybir.AxisListType.X)
                nc.vector.reciprocal(out=r_sb[:, j:j + 1], in_=l_sb[:, j:j + 1])
                nc.vector.tensor_scalar_mul(Psb[:, j, :], t1, r_sb[:, j:j + 1])
                # DMA XBAR transpose: P^T slab [k, 128 own qs]
                nc.sync.dma_start(out=PT[:, :, j * P:(j + 1) * P],
                                  in_=Psb[:, j, :], transpose=True)

            # ---- phase 3: Z^T = (attn @ x)^T  [256 mm] ----------------
            for qc in range(2):
                for dt in range(NT):
                    ps = pp.tile([P, N512], F32, tag="acc")
                    for i in range(NK):
                        nc.tensor.matmul(
                            ps,
                            xrows[:, i, dt * P:(dt + 1) * P],
                            PT[:, i, qc * N512:(qc + 1) * N512],
                            start=(i == 0), stop=(i == NK - 1))
                    nc.vector.tensor_copy(
                        out=gz[:, dt, qc * N512:(qc + 1) * N512], in_=ps)

            # ---- phase 4: vision = Z @ Wv^T + bv  [128 mm] ------------
            for j in range(NQ):
                ev = stage.tile([P, D], BF16, tag="ev")
                for ec in range(2):
                    ps = pp.tile([P, N512], F32, tag="acc")
                    for dt in range(NT):
                        nc.tensor.matmul(
                            ps,
                            gz[:, dt, j * P:(j + 1) * P],
                            mw[:, dt, ec * 4:(ec + 1) * 4, :],
                            start=(dt == 0), stop=(dt == NT - 1))
                    nc.vector.tensor_add(ev[:, ec * N512:(ec + 1) * N512],
                                         ps,
                                         bvb[:, ec * N512:(ec + 1) * N512])
                nc.sync.dma_start(out=vision_h.ap()[j * P:(j + 1) * P, :],
                                  in_=ev)

            # ---- phase 5: text = attn^T @ x_q  [256 mm] ---------------
            for i in range(NK):
                ev = stage.tile([P, D], BF16, tag="ev")
                for dc in range(2):
                    ps = pp.tile([P, N512], F32, tag="acc")
                    for j in range(NQ):
                        nc.tensor.matmul(
                            ps,
                            Psb[:, j, i * P:(i + 1) * P],
                            xrows[:, j, dc * N512:(dc + 1) * N512],
                            start=(j == 0), stop=(j == NQ - 1))
                    nc.vector.tensor_copy(
                        out=ev[:, dc * N512:(dc + 1) * N512], in_=ps)
                nc.scalar.dma_start(out=text_h.ap()[i * P:(i + 1) * P, :],
                                    in_=ev)

    nc.compile()
    return nc


_NC_CACHE = []


def _get_program():
    if not _NC_CACHE:
        _NC_CACHE.append(build_program())
    return _NC_CACHE[0]


def kernel(inputs, Wq, bq, Wk, bk, Wv, bv, _run_opts=None):
    x = np.asarray(inputs, dtype=np.float32)
    Wq = np.asarray(Wq, dtype=np.float32)
    Wk = np.asarray(Wk, dtype=np.float32)
    Wv = np.asarray(Wv, dtype=np.float32)
    bq = np.asarray(bq, dtype=np.float32)
    bk = np.asarray(bk, dtype=np.float32)
    bv = np.asarray(bv, dtype=np.float32)

    M = (Wq.T @ Wk).astype(BF16_NP)              # [d, d']
    # swizzle to [p, et, dt, c]: block et is contiguous per partition
    M_sw = np.ascontiguousarray(
        M.reshape(NT, P, NT, P).transpose(1, 2, 0, 3).reshape(P, D * NT))
    WvT = Wv.T.astype(BF16_NP)                   # [d, e]
    # swizzle to [p, dt, e]: row block dt is contiguous per partition
    WvT_sw = np.ascontiguousarray(
        WvT.reshape(NT, P, D).transpose(1, 0, 2).reshape(P, D * NT))
    w_u = Wq.T @ bk                              # [d]
    w_v = bq @ Wk                                # [d']
    c = float(bq @ bk)
    u_all = (x @ w_u + c) * SCALE                # [B, S]
    phi_all = np.exp((x @ w_v) * SCALE).astype(BF16_NP)

    nc = _get_program()

    in_maps = []
    xt_b, xr_b = {}, {}
    for b in range(B):
        xr_b[b] = x[b].astype(BF16_NP)                          # [S, D]
        xt_b[b] = np.ascontiguousarray(x[b].T).astype(BF16_NP)  # [D, S]
    for core in range(8):
        b, h = divmod(core, 2)
        own = slice(h * SH, (h + 1) * SH)
        oth = slice((1 - h) * SH, (2 - h) * SH)
        xt = np.concatenate([xt_b[b][:, own], xt_b[b][:, oth]], axis=1)
        xr = np.concatenate([xr_b[b][own], xr_b[b][oth]], axis=0)
        phi = np.concatenate([phi_all[b][own], phi_all[b][oth]])
        in_maps.append({
            "xt": np.ascontiguousarray(xt),
            "xr": np.ascontiguousarray(xr),
            "m": M_sw, "wvt": WvT_sw,
            "u": np.ascontiguousarray(u_all[b][own].reshape(NQ, P).T),
            "phi": np.ascontiguousarray(phi),
            "bv": bv,
        })

    run_opts = dict(_run_opts or {})
    res = run_bass_kernel_spmd(nc, in_maps, core_ids=list(range(8)), **run_opts)
    results = res.results

    vision = np.empty((B, S, D), np.float32)
    text = np.zeros((B, S, D), np.float32)
    for core in range(8):
        b, h = divmod(core, 2)
        vision[b, h * SH:(h + 1) * SH] = results[core]["vision"].astype(np.float32)
        tpart = results[core]["text"].astype(np.float32)  # [S, D], [own|other]
        text[b, h * SH:(h + 1) * SH] += tpart[:SH]
        text[b, (1 - h) * SH:(2 - h) * SH] += tpart[SH:]
    if _run_opts is not None:
        return (vision, text), res
    return (vision, text)
